# revision 19
# baseline (speedup 1.0000x reference)
"""GroupedQueryAttention on 8 Trainium2 NeuronCores.

Problem (hardcoded): B=2, T=2048, DIM=4096, 32 q heads, 8 kv heads, hd=128.
  q = x @ Wq.T ; k,v = split(x @ Wkv.T) ; causal softmax(q k^T/sqrt(hd)) v ; out = o @ Wo.T

Sharding: hybrid data x tensor parallel over 8 cores.
  core c -> batch b = c//4, kv-head group j = c%4 (kv heads {2j,2j+1}, q heads {8j..8j+7}).

Per core (v2 - compound-matmul design):
  phase 1: projections with xT resident in SBUF and wall weights streamed as
           [128,128] stationaries; ONE compound matmul per (e-tile, cb) with a
           [128,2048] 4-bank PSUM accumulator -> 1 LDWEIGHTS + 4 MATMULs.
           K/V e-tiles drain to SBUF; Q e-tiles bounce through DRAM (SBUF space).
  phase 2: attention per kv-head g, q-head PAIR: compound sT matmul over both
           heads ([128,1024] psum, double buffered), scalar-engine exp with
           causally trimmed access patterns, softmax denominator accumulated on
           DVE in bf16 (partition-partials; the 128-way f32 sum happens in a
           ones[128,128] broadcast matmul), reciprocal_approx_fast, per-head AV
           matmuls. Per-head AllGather (bf16) fires as each pair completes;
           gathered oT is staged DRAM->SBUF during phase 2.
  phase 3: outT row-slice = WoT.T @ oT_full entirely from SBUF; one compound
           matmul per (oc, eb) with [128,2048] psum double buffered; eb order
           follows AllGather completion order to absorb the collective tail.
Host: output slices are disjoint; just transpose/concat.
"""

import sys

sys.path.insert(0, "/opt/trn_rl_repo")

import math

import numpy as np

import concourse.bass as bass
import concourse.bacc as bacc
import concourse.tile as tile
from concourse import mybir
from concourse.bass_utils import run_bass_kernel_spmd

B, T, DIM = 2, 2048, 4096
N_HEADS, N_KV, HD = 32, 8, 128
R = N_HEADS // N_KV  # 4
NCORES = 8
GROUPS = [[0, 1, 2, 3], [4, 5, 6, 7]]

HPC = 8  # q heads per core
KVPC = 2  # kv heads per core
EQ = HPC * HD  # 1024 q-proj out features per core
EKV = KVPC * HD  # 256 k (and v) out features per core
NT = T // 512  # 4 t-groups of 512
NC = DIM // 128  # 32 contraction tiles
NKB = T // 128  # 16 k-tiles per head

BF = mybir.dt.bfloat16
F32 = mybir.dt.float32
INV_SQRT_HD = 1.0 / math.sqrt(HD)


def build():
    nc = bacc.Bacc("TRN2", num_devices=NCORES)

    # ---- external I/O (per-core data differs, program is SPMD-identical) ----
    xT = nc.dram_tensor("xT", [DIM, T], BF, kind="ExternalInput")  # x[b].T
    wallT = nc.dram_tensor("wallT", [DIM, EQ + 2 * EKV], BF, kind="ExternalInput")
    woT = nc.dram_tensor("woT", [DIM, EQ], BF, kind="ExternalInput")  # Wo[oc_slice,:].T
    maskT = nc.dram_tensor("maskT", [128, 128], BF, kind="ExternalInput")
    ident = nc.dram_tensor("ident", [128, 128], BF, kind="ExternalInput")
    ones128 = nc.dram_tensor("ones128", [128, 128], BF, kind="ExternalInput")
    out_part = nc.dram_tensor("out_part", [EQ, T], F32, kind="ExternalOutput")

    # e-tile column offsets inside wallT: [Q0..Q7 | K0 K1 | V0 V1]
    # processing order: K, V first (attention deps), then Q0..Q7
    e_seq = [HPC, HPC + 1, HPC + 2, HPC + 3] + list(range(HPC))

    with tile.TileContext(nc) as tc:
        with (
            tc.tile_pool(name="persist", bufs=1) as persist,
            tc.tile_pool(name="dram2", bufs=1, space="DRAM") as dram2,
        ):
            # ---------------- constants ----------------
            mask_sb = persist.tile([128, 128], BF)
            nc.sync.dma_start(out=mask_sb[:], in_=maskT[:, :])
            ident_sb = persist.tile([128, 128], BF)
            nc.sync.dma_start(out=ident_sb[:], in_=ident[:, :])
            ones_sb = persist.tile([128, 128], BF)
            nc.sync.dma_start(out=ones_sb[:], in_=ones128[:, :])

            # persistent activations (K and V live in SBUF through phase 2)
            kt_sb = persist.tile([128, KVPC * T], BF)  # KT per kv head
            v_sb = persist.tile([128, KVPC * T], BF)  # V[t,dv] tiles

            # DRAM intermediates
            qt_dram = dram2.tile([128, HPC * T], BF, name="qt_dram")
            og_in = []
            og_out = []
            for h in range(HPC):
                og_in.append(dram2.tile([128, T], BF, name=f"og_in_{h}"))
                og_out.append(dram2.tile([4 * 128, T], BF, name=f"og_out_{h}"))

            # ================= phase 1: projections =================
            with (
                tc.tile_pool(name="vt_pool", bufs=1) as vt_pool,
            ):
                vt_sb = vt_pool.tile([128, KVPC * T], BF)

                with (
                    tc.tile_pool(name="xt_pool", bufs=NC) as xt_pool,
                    tc.tile_pool(name="wstream", bufs=12) as wstream,
                    tc.tile_pool(name="qstage", bufs=2) as qstage,
                    tc.tile_pool(name="p1ps", bufs=2, space="PSUM") as p1ps,
                ):
                    # resident xT: one tile per contraction block so the first
                    # e-sweep starts as soon as cb=0 lands (issue on gpsimd to
                    # keep the sync queue free for the wall-weight stream)
                    xt_sb = []
                    for cb in range(NC):
                        xtt = xt_pool.tile([128, T], BF, tag="xt",
                                           name=f"xt_{cb}")
                        for hh in range(2):
                            nc.gpsimd.dma_start(
                                out=xtt[:, hh * 1024:(hh + 1) * 1024],
                                in_=xT[cb * 128:(cb + 1) * 128,
                                       hh * 1024:(hh + 1) * 1024],
                            )
                        xt_sb.append(xtt)

                    for e in e_seq:
                        acc = p1ps.tile([128, T], F32, tag="acc")
                        for cb in range(NC):
                            wt = wstream.tile([128, 128], BF, tag="w")
                            nc.sync.dma_start(
                                out=wt[:],
                                in_=wallT[cb * 128:(cb + 1) * 128,
                                          e * 128:(e + 1) * 128],
                            )
                            for tg in range(NT):
                                nc.tensor.matmul(
                                    acc[:, tg * 512:(tg + 1) * 512],
                                    wt[:],
                                    xt_sb[cb][:, tg * 512:(tg + 1) * 512],
                                    start=(cb == 0),
                                    stop=(cb == NC - 1),
                                )
                        if e < HPC:  # Q -> stage -> DRAM
                            qs = qstage.tile([128, T], BF, tag="qs")
                            nc.vector.tensor_copy(qs[:], acc[:])
                            nc.sync.dma_start(
                                out=qt_dram[:, e * T:(e + 1) * T], in_=qs[:]
                            )
                        elif e < HPC + KVPC:  # K
                            g = e - HPC
                            nc.vector.tensor_copy(
                                kt_sb[:, g * T:(g + 1) * T], acc[:]
                            )
                        else:  # V
                            g = e - HPC - KVPC
                            nc.vector.tensor_copy(
                                vt_sb[:, g * T:(g + 1) * T], acc[:]
                            )

                # V = VT.T per 128x128 tile (PE transpose; tiny psum pool)
                with tc.tile_pool(name="tpps", bufs=2, space="PSUM") as tpps:
                    for g in range(KVPC):
                        for kb in range(NKB):
                            tp = tpps.tile([128, 128], BF, tag="tp")
                            nc.tensor.transpose(
                                tp[:],
                                vt_sb[:, g * T + kb * 128:g * T + (kb + 1) * 128],
                                ident_sb[:],
                            )
                            nc.vector.tensor_copy(
                                v_sb[:, (g * NKB + kb) * 128:
                                     (g * NKB + kb + 1) * 128],
                                tp[:],
                            )

            # ================= phase 2+3 =================
            with (
                tc.tile_pool(name="og_pool", bufs=1) as og_pool,
            ):
                og_sb = og_pool.tile([128, 4 * HPC * T], BF)  # eb-major

                with (
                    tc.tile_pool(name="qstream", bufs=5) as qstream,
                    tc.tile_pool(name="expp", bufs=3) as expp,
                    tc.tile_pool(name="denp", bufs=2) as denp,
                    tc.tile_pool(name="recipp", bufs=2) as recipp,
                    tc.tile_pool(name="ohead", bufs=3) as ohead,
                    tc.tile_pool(name="ps_sT", bufs=2, space="PSUM") as ps_sT,
                    tc.tile_pool(name="ps_oT", bufs=2, space="PSUM") as ps_oT,
                ):
                    for g in range(KVPC):
                        for pr in range(2):  # head pair within group
                            h0 = g * 4 + pr * 2  # local head index of pair
                            # prefetch q for this pair (2 DMAs per tg)
                            qts_tiles = []
                            for tg in range(NT):
                                qts = qstream.tile([128, 1024], BF, tag="qts")
                                for i in range(2):
                                    nc.gpsimd.dma_start(
                                        out=qts[:, i * 512:(i + 1) * 512],
                                        in_=qt_dram[:, (h0 + i) * T + tg * 512:
                                                    (h0 + i) * T + (tg + 1) * 512],
                                    )
                                qts_tiles.append(qts)

                            oh = [
                                ohead.tile([128, T], BF, tag="oh",
                                           name=f"oh_{g}_{pr}_{i}")
                                for i in range(2)
                            ]
                            for tg in range(NT):
                                nkb = 4 * tg + 4
                                qts = qts_tiles[tg]
                                oT2 = ps_oT.tile([128, 1024], F32, tag="oT")
                                den = denp.tile([128, 1024], BF, tag="den")

                                def emit_av(ex, kb, c0, C):
                                    for i in range(2):
                                        nc.tensor.matmul(
                                            oT2[:, i * 512 + c0:(i + 1) * 512],
                                            v_sb[:, (g * NKB + kb) * 128:
                                                 (g * NKB + kb + 1) * 128],
                                            ex[:, i * C:(i + 1) * C],
                                            start=(kb == 0),
                                            stop=(kb == nkb - 1),
                                            skip_group_check=True,
                                        )

                                pend = None  # software pipeline: AV lags by 1
                                for kb in range(nkb):
                                    jd = kb - 4 * tg  # diag subtile index
                                    c0 = max(jd, 0) * 128  # first valid col
                                    C = 512 - c0
                                    sT = ps_sT.tile([128, 1024], F32, tag="sT")
                                    for i in range(2):
                                        nc.tensor.matmul(
                                            sT[:, i * 512:(i + 1) * 512],
                                            kt_sb[:, g * T + kb * 128:
                                                  g * T + (kb + 1) * 128],
                                            qts[:, i * 512:(i + 1) * 512],
                                            start=True,
                                            stop=(jd < 0),
                                            skip_group_check=True,
                                        )
                                    if 0 <= jd:
                                        # causal mask added in PE: out[k,q] +=
                                        # maskT[q,k] via (maskT)^T @ I
                                        for i in range(2):
                                            nc.tensor.matmul(
                                                sT[:, i * 512 + c0:
                                                   i * 512 + c0 + 128],
                                                mask_sb[:],
                                                ident_sb[:],
                                                start=False,
                                                stop=True,
                                                skip_group_check=True,
                                            )
                                    # exp with causal trim; packed [h0 C | h1 C]
                                    ex = expp.tile([128, 1024], BF, tag="ex")
                                    if C == 512:
                                        nc.scalar.activation(
                                            ex[:, :],
                                            sT[:, :],
                                            mybir.ActivationFunctionType.Exp,
                                            scale=INV_SQRT_HD,
                                        )
                                    else:
                                        nc.scalar.activation(
                                            ex[:, :2 * C].rearrange(
                                                "p (b c) -> p b c", b=2
                                            ),
                                            sT[:, :].rearrange(
                                                "p (b c) -> p b c", b=2
                                            )[:, :, c0:],
                                            mybir.ActivationFunctionType.Exp,
                                            scale=INV_SQRT_HD,
                                        )
                                    # denominator partials accumulate on DVE
                                    if kb == 0:
                                        nc.vector.tensor_copy(den[:, :], ex[:, :])
                                    elif C == 512:
                                        nc.vector.tensor_tensor(
                                            den[:, :], den[:, :], ex[:, :],
                                            mybir.AluOpType.add,
                                        )
                                    else:
                                        den3 = den[:, :].rearrange(
                                            "p (b c) -> p b c", b=2
                                        )[:, :, c0:]
                                        nc.vector.tensor_tensor(
                                            den3,
                                            den3,
                                            ex[:, :2 * C].rearrange(
                                                "p (b c) -> p b c", b=2
                                            ),
                                            mybir.AluOpType.add,
                                        )
                                    if pend is not None:
                                        emit_av(*pend)
                                    pend = (ex, kb, c0, C)
                                emit_av(*pend)
                                # denominator broadcast sum (f32, in PE) + recip
                                den_b = ps_sT.tile([128, 1024], F32, tag="sT")
                                for i in range(2):
                                    nc.tensor.matmul(
                                        den_b[:, i * 512:(i + 1) * 512],
                                        ones_sb[:],
                                        den[:, i * 512:(i + 1) * 512],
                                        start=True, stop=True,
                                    )
                                rc = recipp.tile([128, 1024], F32, tag="rc")
                                nc.vector.reciprocal_approx_fast(rc[:], den_b[:])
                                for i in range(2):
                                    nc.vector.tensor_tensor(
                                        oh[i][:, tg * 512:(tg + 1) * 512],
                                        oT2[:, i * 512:(i + 1) * 512],
                                        rc[:, i * 512:(i + 1) * 512],
                                        mybir.AluOpType.mult,
                                    )
                                    nc.gpsimd.dma_start(
                                        out=og_in[h0 + i][:, tg * 512:(tg + 1) * 512],
                                        in_=oh[i][:, tg * 512:(tg + 1) * 512],
                                    )
                            # ship the pair: AllGather + stage into SBUF
                            for i in range(2):
                                hl = h0 + i
                                nc.gpsimd.collective_compute(
                                    "AllGather",
                                    mybir.AluOpType.bypass,
                                    replica_groups=GROUPS,
                                    ins=[og_in[hl].opt()],
                                    outs=[og_out[hl].opt()],
                                )
                                for r in range(4):
                                    eb = r * HPC + hl
                                    for th in range(2):
                                        nc.gpsimd.dma_start(
                                            out=og_sb[:, eb * T + th * 1024:
                                                      eb * T + (th + 1) * 1024],
                                            in_=og_out[hl][r * 128:(r + 1) * 128,
                                                           th * 1024:(th + 1) * 1024],
                                        )

                # ---------------- phase 3: outT slice = WoT.T @ oT_full ------
                # eb order = AllGather completion order (by local head)
                eb_order = [r * HPC + hl for hl in range(HPC) for r in range(4)]
                with (
                    tc.tile_pool(name="wostream", bufs=8) as wostream,
                    tc.tile_pool(name="ostage", bufs=2) as ostage,
                    tc.tile_pool(name="ps_out", bufs=2, space="PSUM") as ps_out,
                ):
                    for oc in range(HPC):
                        acc = ps_out.tile([128, T], F32, tag="out")
                        for ei, eb in enumerate(eb_order):
                            wt = wostream.tile([128, 128], BF, tag="wo")
                            nc.scalar.dma_start(
                                out=wt[:],
                                in_=woT[eb * 128:(eb + 1) * 128,
                                        oc * 128:(oc + 1) * 128],
                            )
                            for tg in range(NT):
                                nc.tensor.matmul(
                                    acc[:, tg * 512:(tg + 1) * 512],
                                    wt[:],
                                    og_sb[:, eb * T + tg * 512:
                                          eb * T + (tg + 1) * 512],
                                    start=(ei == 0),
                                    stop=(ei == 4 * HPC - 1),
                                )
                        st = ostage.tile([128, T], F32, tag="st")
                        nc.vector.tensor_copy(st[:], acc[:])
                        nc.sync.dma_start(
                            out=out_part[oc * 128:(oc + 1) * 128, :], in_=st[:]
                        )
    nc.finalize()
    return nc


_NC_CACHE = None


def _get_nc():
    global _NC_CACHE
    if _NC_CACHE is None:
        _NC_CACHE = build()
    return _NC_CACHE


def kernel(x, Wq, Wkv, Wo):
    x = np.asarray(x, dtype=np.float32)
    Wq = np.asarray(Wq, dtype=np.float32)
    Wkv = np.asarray(Wkv, dtype=np.float32)
    Wo = np.asarray(Wo, dtype=np.float32)

    # host-side prep (transposes + bf16 casts)
    try:
        import ml_dtypes

        bf16 = ml_dtypes.bfloat16
    except ImportError:  # pragma: no cover
        import jax.numpy as jnp

        bf16 = jnp.bfloat16

    xT_b = [np.ascontiguousarray(x[b].T).astype(bf16) for b in range(B)]

    mask = np.where(
        np.arange(128)[:, None] <= np.arange(128)[None, :], 0.0, -1e30
    ).astype(np.float32)  # [k,q]: allow k<=q
    maskT = np.ascontiguousarray(mask.T).astype(bf16)
    ident = np.eye(128, dtype=np.float32).astype(bf16)
    ones = np.ones((128, 128), dtype=np.float32).astype(bf16)

    in_maps = []
    for c in range(NCORES):
        b, j = c // 4, c % 4
        wq_l = Wq[EQ * j:EQ * (j + 1), :]  # [1024, 4096]
        wk_l = Wkv[EKV * j:EKV * (j + 1), :]  # [256, 4096]
        wv_l = Wkv[N_KV * HD + EKV * j:N_KV * HD + EKV * (j + 1), :]
        wall = np.concatenate([wq_l, wk_l, wv_l], axis=0)  # [1536, 4096]
        wallT = np.ascontiguousarray(wall.T).astype(bf16)  # [4096, 1536]
        woT_l = np.ascontiguousarray(Wo[EQ * j:EQ * (j + 1), :].T).astype(bf16)
        in_maps.append(
            {
                "xT": xT_b[b],
                "wallT": wallT,
                "woT": woT_l,
                "maskT": maskT,
                "ident": ident,
                "ones128": ones,
            }
        )

    nc = _get_nc()
    res = run_bass_kernel_spmd(nc, in_maps, core_ids=list(range(NCORES)))

    out = np.empty((B, T, DIM), dtype=np.float32)
    for b in range(B):
        outT = np.concatenate(
            [res.results[b * 4 + j]["out_part"] for j in range(4)], axis=0
        )  # [4096, 2048]
        out[b] = outT.T
    return out


# revision 21
# speedup vs baseline: 1.1433x; 1.1433x over previous
"""GroupedQueryAttention on 8 Trainium2 NeuronCores.

Problem (hardcoded): B=2, T=2048, DIM=4096, 32 q heads, 8 kv heads, hd=128.
  q = x @ Wq.T ; k,v = split(x @ Wkv.T) ; causal softmax(q k^T/sqrt(hd)) v ; out = o @ Wo.T

Sharding: hybrid data x tensor parallel over 8 cores.
  core c -> batch b = c//4, kv-head group j = c%4 (kv heads {2j,2j+1}, q heads {8j..8j+7}).

Per core (v2 - compound-matmul design):
  phase 1: projections with xT resident in SBUF and wall weights streamed as
           [128,128] stationaries; ONE compound matmul per (e-tile, cb) with a
           [128,2048] 4-bank PSUM accumulator -> 1 LDWEIGHTS + 4 MATMULs.
           K/V e-tiles drain to SBUF; Q e-tiles bounce through DRAM (SBUF space).
  phase 2: attention per kv-head g, q-head PAIR: compound sT matmul over both
           heads ([128,1024] psum, double buffered), scalar-engine exp with
           causally trimmed access patterns, softmax denominator accumulated on
           DVE in bf16 (partition-partials; the 128-way f32 sum happens in a
           ones[128,128] broadcast matmul), reciprocal_approx_fast, per-head AV
           matmuls. Per-head AllGather (bf16) fires as each pair completes;
           gathered oT is staged DRAM->SBUF during phase 2.
  phase 3: outT row-slice = WoT.T @ oT_full entirely from SBUF; one compound
           matmul per (oc, eb) with [128,2048] psum double buffered; eb order
           follows AllGather completion order to absorb the collective tail.
Host: output slices are disjoint; just transpose/concat.
"""

import sys

sys.path.insert(0, "/opt/trn_rl_repo")

import math

import numpy as np

import concourse.bass as bass
import concourse.bacc as bacc
import concourse.tile as tile
from concourse import mybir
from concourse.bass_utils import run_bass_kernel_spmd

B, T, DIM = 2, 2048, 4096
N_HEADS, N_KV, HD = 32, 8, 128
R = N_HEADS // N_KV  # 4
NCORES = 8
GROUPS = [[0, 1, 2, 3], [4, 5, 6, 7]]

HPC = 8  # q heads per core
KVPC = 2  # kv heads per core
EQ = HPC * HD  # 1024 q-proj out features per core
EKV = KVPC * HD  # 256 k (and v) out features per core
NT = T // 512  # 4 t-groups of 512
NC = DIM // 128  # 32 contraction tiles
NKB = T // 128  # 16 k-tiles per head

BF = mybir.dt.bfloat16
F32 = mybir.dt.float32
INV_SQRT_HD = 1.0 / math.sqrt(HD)


def build():
    nc = bacc.Bacc("TRN2", num_devices=NCORES)

    # ---- external I/O (per-core data differs, program is SPMD-identical) ----
    xT = nc.dram_tensor("xT", [DIM, T], BF, kind="ExternalInput")  # x[b].T
    wallT = nc.dram_tensor("wallT", [DIM, EQ + 2 * EKV], BF, kind="ExternalInput")
    woT = nc.dram_tensor("woT", [DIM, EQ], BF, kind="ExternalInput")  # Wo[oc_slice,:].T
    maskT = nc.dram_tensor("maskT", [128, 128], BF, kind="ExternalInput")
    ident = nc.dram_tensor("ident", [128, 128], BF, kind="ExternalInput")
    ones128 = nc.dram_tensor("ones128", [128, 128], BF, kind="ExternalInput")
    out_part = nc.dram_tensor("out_part", [EQ, T], F32, kind="ExternalOutput")

    # e-tile column offsets inside wallT: [Q0..Q7 | K0 K1 | V0 V1]
    # processing order: K, V first (attention deps), then Q0..Q7
    e_seq = [HPC, HPC + 1, HPC + 2, HPC + 3] + list(range(HPC))

    with tile.TileContext(nc) as tc:
        with (
            tc.tile_pool(name="persist", bufs=1) as persist,
            tc.tile_pool(name="dram2", bufs=1, space="DRAM") as dram2,
        ):
            # ---------------- constants ----------------
            mask_sb = persist.tile([128, 128], BF)
            nc.sync.dma_start(out=mask_sb[:], in_=maskT[:, :])
            ident_sb = persist.tile([128, 128], BF)
            nc.sync.dma_start(out=ident_sb[:], in_=ident[:, :])
            ones_sb = persist.tile([128, 128], BF)
            nc.sync.dma_start(out=ones_sb[:], in_=ones128[:, :])

            # persistent activations (K and V live in SBUF through phase 2)
            kt_sb = persist.tile([128, KVPC * T], BF)  # KT per kv head
            v_sb = persist.tile([128, KVPC * T], BF)  # V[t,dv] tiles

            # DRAM intermediates
            qt_dram = dram2.tile([128, HPC * T], BF, name="qt_dram")
            og_in = []
            og_out = []
            for h in range(HPC):
                og_in.append(dram2.tile([128, T], BF, name=f"og_in_{h}"))
                og_out.append(dram2.tile([4 * 128, T], BF, name=f"og_out_{h}"))

            # ================= phase 1: projections =================
            with (
                tc.tile_pool(name="vt_pool", bufs=1) as vt_pool,
            ):
                vt_sb = vt_pool.tile([128, KVPC * T], BF)

                with (
                    tc.tile_pool(name="xt_pool", bufs=NC) as xt_pool,
                    tc.tile_pool(name="wstream", bufs=12) as wstream,
                    tc.tile_pool(name="qstage", bufs=2) as qstage,
                    tc.tile_pool(name="p1ps", bufs=2, space="PSUM") as p1ps,
                ):
                    # resident xT: one tile per contraction block so the first
                    # e-sweep starts as soon as cb=0 lands (issue on gpsimd to
                    # keep the sync queue free for the wall-weight stream)
                    xt_sb = []
                    for cb in range(NC):
                        xtt = xt_pool.tile([128, T], BF, tag="xt",
                                           name=f"xt_{cb}")
                        for hh in range(2):
                            nc.gpsimd.dma_start(
                                out=xtt[:, hh * 1024:(hh + 1) * 1024],
                                in_=xT[cb * 128:(cb + 1) * 128,
                                       hh * 1024:(hh + 1) * 1024],
                            )
                        xt_sb.append(xtt)

                    for e in e_seq:
                        acc = p1ps.tile([128, T], F32, tag="acc")
                        for cb in range(NC):
                            wt = wstream.tile([128, 128], BF, tag="w")
                            nc.sync.dma_start(
                                out=wt[:],
                                in_=wallT[cb * 128:(cb + 1) * 128,
                                          e * 128:(e + 1) * 128],
                            )
                            for tg in range(NT):
                                nc.tensor.matmul(
                                    acc[:, tg * 512:(tg + 1) * 512],
                                    wt[:],
                                    xt_sb[cb][:, tg * 512:(tg + 1) * 512],
                                    start=(cb == 0),
                                    stop=(cb == NC - 1),
                                )
                        if e < HPC:  # Q -> stage -> DRAM
                            qs = qstage.tile([128, T], BF, tag="qs")
                            nc.vector.tensor_copy(qs[:], acc[:])
                            nc.sync.dma_start(
                                out=qt_dram[:, e * T:(e + 1) * T], in_=qs[:]
                            )
                        elif e < HPC + KVPC:  # K
                            g = e - HPC
                            nc.vector.tensor_copy(
                                kt_sb[:, g * T:(g + 1) * T], acc[:]
                            )
                        else:  # V
                            g = e - HPC - KVPC
                            nc.vector.tensor_copy(
                                vt_sb[:, g * T:(g + 1) * T], acc[:]
                            )

                # V = VT.T per 128x128 tile (PE transpose; tiny psum pool)
                with tc.tile_pool(name="tpps", bufs=2, space="PSUM") as tpps:
                    for g in range(KVPC):
                        for kb in range(NKB):
                            tp = tpps.tile([128, 128], BF, tag="tp")
                            nc.tensor.transpose(
                                tp[:],
                                vt_sb[:, g * T + kb * 128:g * T + (kb + 1) * 128],
                                ident_sb[:],
                            )
                            nc.vector.tensor_copy(
                                v_sb[:, (g * NKB + kb) * 128:
                                     (g * NKB + kb + 1) * 128],
                                tp[:],
                            )

            # ================= phase 2+3 =================
            with (
                tc.tile_pool(name="og_pool", bufs=1) as og_pool,
            ):
                og_sb = og_pool.tile([128, 4 * HPC * T], BF)  # eb-major

                with (
                    tc.tile_pool(name="qstream", bufs=5) as qstream,
                    tc.tile_pool(name="expp", bufs=3) as expp,
                    tc.tile_pool(name="denp", bufs=2) as denp,
                    tc.tile_pool(name="recipp", bufs=2) as recipp,
                    tc.tile_pool(name="ohead", bufs=3) as ohead,
                    tc.tile_pool(name="ps_sT", bufs=2, space="PSUM") as ps_sT,
                    tc.tile_pool(name="ps_oT", bufs=2, space="PSUM") as ps_oT,
                ):
                    for g in range(KVPC):
                        for pr in range(2):  # head pair within group
                            h0 = g * 4 + pr * 2  # local head index of pair
                            # prefetch q for this pair (2 DMAs per tg)
                            qts_tiles = []
                            for tg in range(NT):
                                qts = qstream.tile([128, 1024], BF, tag="qts")
                                for i in range(2):
                                    nc.sync.dma_start(
                                        out=qts[:, i * 512:(i + 1) * 512],
                                        in_=qt_dram[:, (h0 + i) * T + tg * 512:
                                                    (h0 + i) * T + (tg + 1) * 512],
                                    )
                                qts_tiles.append(qts)

                            oh = [
                                ohead.tile([128, T], BF, tag="oh",
                                           name=f"oh_{g}_{pr}_{i}")
                                for i in range(2)
                            ]
                            for tg in range(NT):
                                nkb = 4 * tg + 4
                                qts = qts_tiles[tg]
                                oT2 = ps_oT.tile([128, 1024], F32, tag="oT")
                                den = denp.tile([128, 1024], BF, tag="den")

                                def emit_av(ex, kb, c0, C):
                                    for i in range(2):
                                        nc.tensor.matmul(
                                            oT2[:, i * 512 + c0:(i + 1) * 512],
                                            v_sb[:, (g * NKB + kb) * 128:
                                                 (g * NKB + kb + 1) * 128],
                                            ex[:, i * C:(i + 1) * C],
                                            start=(kb == 0),
                                            stop=(kb == nkb - 1),
                                            skip_group_check=True,
                                        )

                                pend = None  # software pipeline: AV lags by 1
                                for kb in range(nkb):
                                    jd = kb - 4 * tg  # diag subtile index
                                    c0 = max(jd, 0) * 128  # first valid col
                                    C = 512 - c0
                                    sT = ps_sT.tile([128, 1024], F32, tag="sT")
                                    for i in range(2):
                                        nc.tensor.matmul(
                                            sT[:, i * 512:(i + 1) * 512],
                                            kt_sb[:, g * T + kb * 128:
                                                  g * T + (kb + 1) * 128],
                                            qts[:, i * 512:(i + 1) * 512],
                                            start=True,
                                            stop=(jd < 0),
                                            skip_group_check=True,
                                        )
                                    if 0 <= jd:
                                        # causal mask added in PE: out[k,q] +=
                                        # maskT[q,k] via (maskT)^T @ I
                                        for i in range(2):
                                            nc.tensor.matmul(
                                                sT[:, i * 512 + c0:
                                                   i * 512 + c0 + 128],
                                                mask_sb[:],
                                                ident_sb[:],
                                                start=False,
                                                stop=True,
                                                skip_group_check=True,
                                            )
                                    # exp with causal trim; packed [h0 C | h1 C]
                                    ex = expp.tile([128, 1024], BF, tag="ex")
                                    if C == 512:
                                        nc.scalar.activation(
                                            ex[:, :],
                                            sT[:, :],
                                            mybir.ActivationFunctionType.Exp,
                                            scale=INV_SQRT_HD,
                                        )
                                    else:
                                        nc.scalar.activation(
                                            ex[:, :2 * C].rearrange(
                                                "p (b c) -> p b c", b=2
                                            ),
                                            sT[:, :].rearrange(
                                                "p (b c) -> p b c", b=2
                                            )[:, :, c0:],
                                            mybir.ActivationFunctionType.Exp,
                                            scale=INV_SQRT_HD,
                                        )
                                    # denominator partials accumulate on DVE
                                    if kb == 0:
                                        nc.vector.tensor_copy(den[:, :], ex[:, :])
                                    elif C == 512:
                                        nc.vector.tensor_tensor(
                                            den[:, :], den[:, :], ex[:, :],
                                            mybir.AluOpType.add,
                                        )
                                    else:
                                        den3 = den[:, :].rearrange(
                                            "p (b c) -> p b c", b=2
                                        )[:, :, c0:]
                                        nc.vector.tensor_tensor(
                                            den3,
                                            den3,
                                            ex[:, :2 * C].rearrange(
                                                "p (b c) -> p b c", b=2
                                            ),
                                            mybir.AluOpType.add,
                                        )
                                    if pend is not None:
                                        emit_av(*pend)
                                    pend = (ex, kb, c0, C)
                                emit_av(*pend)
                                # denominator broadcast sum (f32, in PE) + recip
                                den_b = ps_sT.tile([128, 1024], F32, tag="sT")
                                for i in range(2):
                                    nc.tensor.matmul(
                                        den_b[:, i * 512:(i + 1) * 512],
                                        ones_sb[:],
                                        den[:, i * 512:(i + 1) * 512],
                                        start=True, stop=True,
                                    )
                                rc = recipp.tile([128, 1024], F32, tag="rc")
                                nc.vector.reciprocal_approx_fast(rc[:], den_b[:])
                                for i in range(2):
                                    nc.vector.tensor_tensor(
                                        oh[i][:, tg * 512:(tg + 1) * 512],
                                        oT2[:, i * 512:(i + 1) * 512],
                                        rc[:, i * 512:(i + 1) * 512],
                                        mybir.AluOpType.mult,
                                    )
                                    nc.sync.dma_start(
                                        out=og_in[h0 + i][:, tg * 512:(tg + 1) * 512],
                                        in_=oh[i][:, tg * 512:(tg + 1) * 512],
                                    )
                            # ship the pair: AllGather + stage into SBUF
                            for i in range(2):
                                hl = h0 + i
                                nc.gpsimd.collective_compute(
                                    "AllGather",
                                    mybir.AluOpType.bypass,
                                    replica_groups=GROUPS,
                                    ins=[og_in[hl].opt()],
                                    outs=[og_out[hl].opt()],
                                )
                                for r in range(4):
                                    eb = r * HPC + hl
                                    for th in range(2):
                                        nc.gpsimd.dma_start(
                                            out=og_sb[:, eb * T + th * 1024:
                                                      eb * T + (th + 1) * 1024],
                                            in_=og_out[hl][r * 128:(r + 1) * 128,
                                                           th * 1024:(th + 1) * 1024],
                                        )

                # ---------------- phase 3: outT slice = WoT.T @ oT_full ------
                # eb order = AllGather completion order (by local head)
                eb_order = [r * HPC + hl for hl in range(HPC) for r in range(4)]
                with (
                    tc.tile_pool(name="wostream", bufs=8) as wostream,
                    tc.tile_pool(name="ostage", bufs=2) as ostage,
                    tc.tile_pool(name="ps_out", bufs=2, space="PSUM") as ps_out,
                ):
                    for oc in range(HPC):
                        acc = ps_out.tile([128, T], F32, tag="out")
                        for ei, eb in enumerate(eb_order):
                            wt = wostream.tile([128, 128], BF, tag="wo")
                            nc.scalar.dma_start(
                                out=wt[:],
                                in_=woT[eb * 128:(eb + 1) * 128,
                                        oc * 128:(oc + 1) * 128],
                            )
                            for tg in range(NT):
                                nc.tensor.matmul(
                                    acc[:, tg * 512:(tg + 1) * 512],
                                    wt[:],
                                    og_sb[:, eb * T + tg * 512:
                                          eb * T + (tg + 1) * 512],
                                    start=(ei == 0),
                                    stop=(ei == 4 * HPC - 1),
                                )
                        st = ostage.tile([128, T], F32, tag="st")
                        nc.vector.tensor_copy(st[:], acc[:])
                        nc.sync.dma_start(
                            out=out_part[oc * 128:(oc + 1) * 128, :], in_=st[:]
                        )
    nc.finalize()
    return nc


_NC_CACHE = None


def _get_nc():
    global _NC_CACHE
    if _NC_CACHE is None:
        _NC_CACHE = build()
    return _NC_CACHE


def kernel(x, Wq, Wkv, Wo):
    x = np.asarray(x, dtype=np.float32)
    Wq = np.asarray(Wq, dtype=np.float32)
    Wkv = np.asarray(Wkv, dtype=np.float32)
    Wo = np.asarray(Wo, dtype=np.float32)

    # host-side prep (transposes + bf16 casts)
    try:
        import ml_dtypes

        bf16 = ml_dtypes.bfloat16
    except ImportError:  # pragma: no cover
        import jax.numpy as jnp

        bf16 = jnp.bfloat16

    xT_b = [np.ascontiguousarray(x[b].T).astype(bf16) for b in range(B)]

    mask = np.where(
        np.arange(128)[:, None] <= np.arange(128)[None, :], 0.0, -1e30
    ).astype(np.float32)  # [k,q]: allow k<=q
    maskT = np.ascontiguousarray(mask.T).astype(bf16)
    ident = np.eye(128, dtype=np.float32).astype(bf16)
    ones = np.ones((128, 128), dtype=np.float32).astype(bf16)

    in_maps = []
    for c in range(NCORES):
        b, j = c // 4, c % 4
        wq_l = Wq[EQ * j:EQ * (j + 1), :]  # [1024, 4096]
        wk_l = Wkv[EKV * j:EKV * (j + 1), :]  # [256, 4096]
        wv_l = Wkv[N_KV * HD + EKV * j:N_KV * HD + EKV * (j + 1), :]
        wall = np.concatenate([wq_l, wk_l, wv_l], axis=0)  # [1536, 4096]
        wallT = np.ascontiguousarray(wall.T).astype(bf16)  # [4096, 1536]
        woT_l = np.ascontiguousarray(Wo[EQ * j:EQ * (j + 1), :].T).astype(bf16)
        in_maps.append(
            {
                "xT": xT_b[b],
                "wallT": wallT,
                "woT": woT_l,
                "maskT": maskT,
                "ident": ident,
                "ones128": ones,
            }
        )

    nc = _get_nc()
    res = run_bass_kernel_spmd(nc, in_maps, core_ids=list(range(NCORES)))

    out = np.empty((B, T, DIM), dtype=np.float32)
    for b in range(B):
        outT = np.concatenate(
            [res.results[b * 4 + j]["out_part"] for j in range(4)], axis=0
        )  # [4096, 2048]
        out[b] = outT.T
    return out
